# revision 1
# baseline (speedup 1.0000x reference)
"""Trainium2 Bass kernel for nn_Attention_54580444397738 (gnn_message_passing).

Math per batch b (B=8, N=128, H=256, C=16):
  proj         = local @ W_apair                                     [N, H]
  pre[i,j,:]   = proj[i,:] + proj[j,:] + binary[i,j,:] @ W_binary
                 + b_apair + b_binary                                [N, N, H]
  score[i,j]   = sigmoid(relu(pre[i,j,:]) . W_att + b_att)           [N, N]
  glob         = score @ local                                       [N, H]
  local_pair [i,j,:] = local[i,:] + local[j,:]                       (output 1)
  global_pair[i,j,:] = glob[i,:]  + glob[j,:]                        (output 2)

Key algebraic simplification: einsum("bijh,hk->bijk", local_pair, W_apair)
= proj[i,:] + proj[j,:], so the N^2xHxH matmul collapses to an NxHxH one.

Sharding: data-parallel over batch B across the 8 cores (1 batch per core).
The outputs (2 x 16 MB fp32 per core) dominate -> memory-bound.

Implementation notes:
  - All attention matmuls use float32r (TF32-like rounding ~1.2e-4, but
    1 cycle/row vs 4 for fp32 on the PE); the rounding noise attenuates
    through sigmoid to ~1e-4 relative on global_pair. local_pair is exact.
  - Per row-block i, pre[j,:] accumulates in one PSUM bank pair-wise
    (i, i+1): ones@projFlat-row + I@projW + binT@Wx4, where binT comes from
    PE-transposing binary loaded as [j, (i, c-pad32)] (c=16 carries a ones
    lane so the bias rides Wx4 row 16). ACT applies relu; the fused DVE
    custom op affine_mul_reduce computes the W_att dot product into logits
    columns; sigmoid(+b_att) gives scoreT, one matmul gives glob.
  - Output tiles [j=128, h=256] = X[j,:] + X[i,:] are built EXACTLY via a
    compensated f32r pair (xR = f32r(x), xE = f32r(x - xR), error ~1e-8):
    variants per tile position balance the engines:
      'D' pair: 2 N=512 f32r row-matmuls -> PSUM, fused DVE add (X free-dim
          broadcast) writes two tiles at once;
      'E' pair: 4 matmuls build both full tiles in PSUM, ACT copies out;
      'G': GPSIMD partition_broadcast of the exact row + DVE add;
      'C': 2 row-matmuls + DVE add (single tile).
    Tiles stage 8-up in SBUF and store with 1 MB DMAs (1 KB descriptors).
  - Known HW quirks honored here: f32r producers must write f32r dtype;
    matmul operands need 32-aligned base partitions (Wx4/binT replicated at
    {0,32,64,96}); partition_broadcast only reads partition-0 sources;
    reordering matmuls within a PSUM accumulation group can crash the
    device (keep the validated interleaving).
"""

import numpy as np

B, N, H, BIN = 8, 128, 256, 16
NCORES = 8
CPAD = 32        # c dim padded 16 -> 32 so transposed blocks land 32-aligned
IG = 4           # i's per binary-transpose group (4 * 32 = 128)
STAGE_I = 8      # output tiles per staged 1MB DMA store
PROLOG = 16      # local_pair tiles emitted before attention work starts

# variant per tile (A: PE-psum+ACT-copy, C: PE-row-psum+DVE-add,
# G: POOL-bcast+DVE-add). G positions are arithmetic (i%16 = 2+3k) so all
# G rows of a phase load with a single strided DMA.
VAR_PAT = "DdGEeGDdGEeGDdGC"
SKIP_ATTN = False  # probe knob: drop attention/score work (wrong gp values)


def variant_of(i):
    return VAR_PAT[i % 16]

_cache = {}


def _body(tc, io, reps=1):
    import concourse.bass as bass
    import concourse.mybir as mybir
    from concourse.masks import make_identity
    from contextlib import ExitStack, nullcontext

    nc = tc.nc
    ts = bass.ts
    f32 = mybir.dt.float32
    f32r = mybir.dt.float32r
    Relu = mybir.ActivationFunctionType.Relu
    Sigmoid = mybir.ActivationFunctionType.Sigmoid

    local_d, binary_d, wap_d, bap_d, wbin_d, bbin_d, watt_d, batt_d, lp_d, gp_d = io

    ctx = ExitStack()
    with ctx:
        persist = ctx.enter_context(tc.tile_pool(name="persist", bufs=1))
        binTp = ctx.enter_context(tc.tile_pool(name="binTp", bufs=6))
        att2p = ctx.enter_context(tc.tile_pool(name="att2p", bufs=4))
        stagep = ctx.enter_context(tc.tile_pool(name="stagep", bufs=2))
        bcastp = ctx.enter_context(tc.tile_pool(name="bcastp", bufs=3))
        prep = ctx.enter_context(tc.tile_pool(name="prep", bufs=4, space="PSUM"))
        outpp = ctx.enter_context(tc.tile_pool(name="outpp", bufs=4, space="PSUM"))
        dramp = ctx.enter_context(tc.tile_pool(name="dramp", bufs=1, space="DRAM"))

        # timing builds wrap the whole body in a device-side loop
        loop = tc.For_i(0, reps, 1) if reps > 1 else nullcontext()
        ctx.enter_context(loop)

        # ---------------- persistent setup ----------------
        identity = persist.tile([128, 128], f32, tag="identity")
        make_identity(nc, identity)
        identR = persist.tile([128, 128], f32r, tag="identR")
        nc.vector.tensor_copy(out=identR, in_=identity)
        onesF = persist.tile([128, 128], f32, tag="onesF")
        nc.gpsimd.memset(onesF, 1.0)
        onesT = persist.tile([128, 128], f32r, tag="onesT")
        nc.vector.tensor_copy(out=onesT, in_=onesF)

        localSb = persist.tile([N, H], f32, tag="localSb")
        nc.sync.dma_start(out=localSb, in_=local_d)

        # f32r weights (cast during SWDGE load)
        wap0 = persist.tile([128, H], f32r, tag="wap0")
        nc.gpsimd.dma_start(out=wap0, in_=wap_d[0:128])
        wap1 = persist.tile([128, H], f32r, tag="wap1")
        nc.gpsimd.dma_start(out=wap1, in_=wap_d[128:256])

        biasA = persist.tile([1, H], f32, tag="biasA")
        nc.sync.dma_start(out=biasA, in_=bap_d.unsqueeze(0))
        biasB = persist.tile([1, H], f32, tag="biasB")
        nc.sync.dma_start(out=biasB, in_=bbin_d.unsqueeze(0))
        biasRow = persist.tile([1, H], f32r, tag="biasRow")
        nc.vector.tensor_add(out=biasRow, in0=biasA, in1=biasB)

        wbinR = persist.tile([16, H], f32r, tag="wbinR")
        nc.gpsimd.dma_start(out=wbinR, in_=wbin_d)

        wattRow = persist.tile([1, H], f32, tag="wattRow")
        nc.sync.dma_start(out=wattRow, in_=watt_d.rearrange("k o -> o k"))
        battRow = persist.tile([1, 1], f32, tag="battRow")
        nc.sync.dma_start(out=battRow, in_=batt_d.unsqueeze(0))

        # Wx4: W_binary + bias row replicated at partitions {0,32,64,96}
        wx4 = persist.tile([128, H], f32r, tag="wx4")
        for m in range(4):
            nc.sync.dma_start(out=wx4[32 * m : 32 * m + 16, :], in_=wbinR)
            nc.sync.dma_start(out=wx4[32 * m + 16 : 32 * m + 17, :], in_=biasRow)

        # broadcast W_att across partitions; b_att as a [128,1] column
        wattB = persist.tile([128, H], f32, tag="wattB")
        battCol = persist.tile([128, 1], f32, tag="battCol")
        nc.gpsimd.partition_broadcast(wattB, wattRow)
        nc.gpsimd.partition_broadcast(battCol, battRow)

        # localT = local^T (f32r), then projW = local @ W_apair (f32r)
        localT = persist.tile([128, H], f32r, tag="localT")
        for hb in range(2):
            tp = outpp.tile([128, H], f32, tag="outp")
            nc.tensor.transpose(tp[:, 0:128], localSb[:, ts(hb, 128)], identity)
            nc.scalar.copy(out=localT[:, ts(hb, 128)], in_=tp[:, 0:128])
        pp = outpp.tile([128, H], f32, tag="outp")
        nc.tensor.matmul(pp, lhsT=localT[:, 0:128], rhs=wap0, start=True, stop=False)
        nc.tensor.matmul(pp, lhsT=localT[:, 128:256], rhs=wap1, start=False, stop=True)
        projW = persist.tile([128, H], f32r, tag="projW")
        nc.scalar.copy(out=projW, in_=pp)

        projDram = dramp.tile([N, H], f32r, tag="projDram")
        nc.sync.dma_start(out=projDram, in_=projW)
        projFlat = persist.tile([97, 32 * H], f32r, tag="projFlat")
        pf4 = projDram.rearrange("(a x) h -> a (x h)", a=4)
        for q in range(4):
            nc.sync.dma_start(out=projFlat[32 * q : 32 * q + 1, :], in_=pf4[q : q + 1])

        # compensated f32r split of X for exact PE broadcasts:
        #   XR = f32r(X), XE = f32r(X - XR);  XR + XE == X to ~1e-8
        # flatX holds exact f32 rows (for the G-variant POOL broadcast).
        xR = persist.tile([N, H], f32r, tag="xR")
        xE = persist.tile([N, H], f32r, tag="xE")
        flatR = persist.tile([97, 32 * H], f32r, tag="flatR")
        flatE = persist.tile([97, 32 * H], f32r, tag="flatE")
        xRDram = dramp.tile([N, H], f32r, tag="xRDram")
        xEDram = dramp.tile([N, H], f32r, tag="xEDram")

        def split_x(xSb):
            """fill xR/xE from xSb and bounce rows into flatR/flatE"""
            nc.vector.tensor_copy(out=xR, in_=xSb)
            nc.vector.tensor_sub(out=xE, in0=xSb, in1=xR.bitcast(f32))
            nc.sync.dma_start(out=xRDram, in_=xR)
            nc.sync.dma_start(out=xEDram, in_=xE)
            r4 = xRDram.rearrange("(a x) h -> a (x h)", a=4)
            e4 = xEDram.rearrange("(a x) h -> a (x h)", a=4)
            for q in range(4):
                nc.sync.dma_start(out=flatR[32 * q : 32 * q + 1, :], in_=r4[q : q + 1])
                nc.sync.dma_start(out=flatE[32 * q : 32 * q + 1, :], in_=e4[q : q + 1])

        split_x(localSb)

        # binp[j, (i, c32)]: c 0..15 = binary[., i, j, .], c16 = 1.0 (bias lane)
        binp = persist.tile([128, N * CPAD], f32, tag="binp")
        nc.gpsimd.memset(binp, 0.0)
        binp3 = binp.rearrange("p (i c) -> p i c", c=CPAD)
        nc.gpsimd.memset(binp3[:, :, 16:17], 1.0)
        for q in range(4):
            nc.sync.dma_start(
                out=binp3[:, ts(q, 32), 0:BIN],
                in_=binary_d[ts(q, 32)].rearrange("i j c -> j i c"),
            )

        logits = persist.tile([128, N], f32, tag="logits")
        ttrS = persist.tile([128, H], f32, tag="ttrS")
        binT = {}

        # ---------------- helpers ----------------
        def row_rhs(flat, i, width):
            q, r = divmod(i, 32)
            return flat[32 * q : 32 * q + 1, r * H : r * H + width]

        def row_lhsT(i):
            q = i // 32
            return onesT[32 * q : 32 * q + 1, :]

        def row_tp(i):
            return (32 * (i // 32), 0)

        gRows = persist.tile([1, 40 * H], f32, tag="gRows")

        def load_g_rows(x_dram):
            # rows i%16 in {2,5,8,11,14}: offset 2*H, strides (16*H, 3*H)
            src = x_dram.rearrange("n h -> (n h)")
            src3 = bass.AP(src.tensor, src.offset + 2 * H,
                           [[16 * H, 8], [3 * H, 5], [1, H]])
            nc.sync.dma_start(out=gRows.rearrange("o (a b h) -> o a b h", a=8, b=5),
                              in_=src3.unsqueeze(0))

        def g_row(i):
            q, r = divmod(i, 16)
            g = q * 5 + (r - 2) // 3
            return gRows[0:1, g * H : (g + 1) * H]

        def out_tile_pair(i, xSb, stage):
            # tiles i, i+1 in one psum bank: rows via N=512 f32r pair-MMs,
            # one fused DVE add with free-broadcast of the X tile.
            dst = stage[:, (i % STAGE_I) * H : (i % STAGE_I) * H + 2 * H]
            po = prep.tile([128, 2 * H], f32, tag="pre")
            nc.tensor.matmul(po, lhsT=row_lhsT(i), rhs=row_rhs(flatR, i, 2 * H),
                             start=True, stop=False, tile_position=row_tp(i))
            nc.tensor.matmul(po, lhsT=row_lhsT(i), rhs=row_rhs(flatE, i, 2 * H),
                             start=False, stop=True, tile_position=row_tp(i))
            nc.vector.tensor_add(
                out=dst, in0=xSb.unsqueeze(1).broadcast_to([128, 2, H]), in1=po)

        def out_tile_pairA(i, xSb, stage):
            # tiles i, i+1 via full psum pair + ACT copy (broadcast-rhs MMs)
            dst = stage[:, (i % STAGE_I) * H : (i % STAGE_I) * H + 2 * H]
            po = prep.tile([128, 2 * H], f32, tag="pre")
            xr2 = xR.unsqueeze(1).broadcast_to([128, 2, H])
            xe2 = xE.unsqueeze(1).broadcast_to([128, 2, H])
            nc.tensor.matmul(po, lhsT=identR, rhs=xr2, start=True, stop=False)
            nc.tensor.matmul(po, lhsT=identR, rhs=xe2, start=False, stop=False)
            nc.tensor.matmul(po, lhsT=row_lhsT(i), rhs=row_rhs(flatR, i, 2 * H),
                             start=False, stop=False, tile_position=row_tp(i))
            nc.tensor.matmul(po, lhsT=row_lhsT(i), rhs=row_rhs(flatE, i, 2 * H),
                             start=False, stop=True, tile_position=row_tp(i))
            nc.scalar.copy(out=dst, in_=po)

        def out_tile(i, xSb, x_dram, stage, variant):
            dst = stage[:, ts(i % STAGE_I, H)]
            if variant in ("A", "B"):
                po = outpp.tile([128, H], f32, tag="outp")
                nc.tensor.matmul(po, lhsT=identR, rhs=xR, start=True, stop=False)
                nc.tensor.matmul(po, lhsT=identR, rhs=xE, start=False, stop=False)
                nc.tensor.matmul(po, lhsT=row_lhsT(i), rhs=row_rhs(flatR, i, H),
                                 start=False, stop=False, tile_position=row_tp(i))
                nc.tensor.matmul(po, lhsT=row_lhsT(i), rhs=row_rhs(flatE, i, H),
                                 start=False, stop=True, tile_position=row_tp(i))
                if variant == "A":
                    nc.scalar.copy(out=dst, in_=po)
                else:
                    nc.vector.tensor_copy(out=dst, in_=po)
            elif variant == "C":
                po = outpp.tile([128, H], f32, tag="outp")
                nc.tensor.matmul(po, lhsT=row_lhsT(i), rhs=row_rhs(flatR, i, H),
                                 start=True, stop=False, tile_position=row_tp(i))
                nc.tensor.matmul(po, lhsT=row_lhsT(i), rhs=row_rhs(flatE, i, H),
                                 start=False, stop=True, tile_position=row_tp(i))
                nc.vector.tensor_add(out=dst, in0=xSb, in1=po)
            else:  # 'G': bit-exact POOL broadcast + DVE add
                bt = bcastp.tile([128, H], f32, tag="bc")
                nc.gpsimd.partition_broadcast(bt, g_row(i))
                nc.vector.tensor_add(out=dst, in0=xSb, in1=bt)

        def flush_stage(i0, stage, dram_out):
            nc.sync.dma_start(
                out=dram_out[i0 : i0 + STAGE_I].rearrange("i j h -> j i h"),
                in_=stage.rearrange("p (i h) -> p i h", h=H),
            )

        def attn_step(i):
            g, il = divmod(i, IG)
            if il == 0:  # transpose this binary group: [j,(i4,c32)] -> [(i4,c32),j]
                tp = outpp.tile([128, H], f32, tag="outp")
                nc.tensor.transpose(tp[:, 0:128], binp[:, ts(g, 128)], identity)
                bt = binTp.tile([128, 128], f32r, tag="binT")
                nc.scalar.copy(out=bt, in_=tp[:, 0:128])
                binT[g] = bt
            if i % 2 == 1:
                return
            pre = prep.tile([128, 2 * H], f32, tag="pre")
            nc.tensor.matmul(pre, lhsT=row_lhsT(i), rhs=row_rhs(projFlat, i, 2 * H),
                             start=True, stop=False, tile_position=row_tp(i))
            for m in range(2):
                ii = i + m
                gg, iil = divmod(ii, IG)
                nc.tensor.matmul(pre[:, ts(m, H)], lhsT=identR, rhs=projW,
                                 start=False, stop=False)
                nc.tensor.matmul(
                    pre[:, ts(m, H)],
                    lhsT=binT[gg][32 * iil : 32 * iil + 17, :],
                    rhs=wx4[32 * iil : 32 * iil + 17, :],
                    start=False, stop=(m == 1), tile_position=(32 * iil, 0),
                )
            a2 = att2p.tile([128, 2 * H], f32, tag="att2")
            nc.scalar.activation(out=a2, in_=pre, func=Relu)
            for m in range(2):
                nc.vector.affine_mul_reduce(
                    out=ttrS, accum_out=logits[:, i + m : i + m + 1],
                    in0=a2[:, ts(m, H)], in1=wattB, scale=1.0, bias=0.0,
                )

        # ---------------- phase 1: local_pair + attention ----------------
        def out_phase(xSb, x_dram, dram_out, with_attn):
            stage = None
            i = 0
            attn_at = 0

            def attn_tick(limit):
                nonlocal attn_at
                if with_attn:
                    while attn_at < min(limit, N):
                        attn_step(attn_at)
                        attn_at += 1

            while i < N:
                if i % STAGE_I == 0:
                    stage = stagep.tile([128, STAGE_I * H], f32, tag="stage")
                v = variant_of(i)
                if v == "D":
                    out_tile_pair(i, xSb, stage)
                    step = 2
                elif v == "E":
                    out_tile_pairA(i, xSb, stage)
                    step = 2
                else:
                    out_tile(i, xSb, x_dram, stage, v)
                    step = 1
                for k in range(step):
                    if (i + k) % STAGE_I == STAGE_I - 1:
                        flush_stage(i + k - STAGE_I + 1, stage, dram_out)
                i += step
                attn_tick(i - PROLOG)
            attn_tick(N)

        load_g_rows(local_d)
        out_phase(localSb, local_d, lp_d, with_attn=not SKIP_ATTN)

        # ---------------- scores -> glob ----------------
        scoreT = persist.tile([128, N], f32, tag="scoreT")
        globSb = persist.tile([128, H], f32, tag="globSb")
        if SKIP_ATTN:
            nc.vector.tensor_copy(out=globSb, in_=localSb)
        else:
            nc.scalar.activation(out=scoreT, in_=logits, func=Sigmoid, bias=battCol)
            pg = outpp.tile([128, H], f32, tag="outp")
            nc.tensor.matmul(pg, lhsT=scoreT, rhs=localSb, start=True, stop=True)
            nc.vector.tensor_copy(out=globSb, in_=pg)
        globDram = dramp.tile([N, H], f32, tag="globDram")
        nc.sync.dma_start(out=globDram, in_=globSb)
        split_x(globSb)
        load_g_rows(globDram)

        # ---------------- phase 2: global_pair ----------------
        out_phase(globSb, globDram, gp_d, with_attn=False)


def _build(reps=1):
    import concourse.bass as bass  # noqa: F401
    from concourse import bacc
    import concourse.mybir as mybir
    import concourse.tile as tile

    f32 = mybir.dt.float32
    nc = bacc.Bacc(
        "TRN2",
        target_bir_lowering=False,
        debug=False,
        enable_asserts=False,
        num_devices=NCORES,
    )
    io = (
        nc.dram_tensor("local", [N, H], f32, kind="ExternalInput").ap(),
        nc.dram_tensor("binary", [N, N, BIN], f32, kind="ExternalInput").ap(),
        nc.dram_tensor("w_apair", [H, H], f32, kind="ExternalInput").ap(),
        nc.dram_tensor("b_apair", [H], f32, kind="ExternalInput").ap(),
        nc.dram_tensor("w_binary", [BIN, H], f32, kind="ExternalInput").ap(),
        nc.dram_tensor("b_binary", [H], f32, kind="ExternalInput").ap(),
        nc.dram_tensor("w_att", [H, 1], f32, kind="ExternalInput").ap(),
        nc.dram_tensor("b_att", [1], f32, kind="ExternalInput").ap(),
        nc.dram_tensor("out_lp", [N, N, H], f32, kind="ExternalOutput").ap(),
        nc.dram_tensor("out_gp", [N, N, H], f32, kind="ExternalOutput").ap(),
    )
    with tile.TileContext(nc) as tc:
        _body(tc, io, reps=reps)
    nc.compile()
    return nc


def _get_nc():
    if "nc" not in _cache:
        _cache["nc"] = _build()
    return _cache["nc"]


def _run(inputs, trace=False):
    from concourse.bass_utils import run_bass_kernel_spmd

    nc = _get_nc()
    f = lambda x: np.ascontiguousarray(np.asarray(x), dtype=np.float32)
    shared = {
        "w_apair": f(inputs["W_apair"]),
        "b_apair": f(inputs["b_apair"]),
        "w_binary": f(inputs["W_binary"]),
        "b_binary": f(inputs["b_binary"]),
        "w_att": f(inputs["W_att"]),
        "b_att": f(inputs["b_att"]),
    }
    local = f(inputs["local_feats"])
    binary = f(inputs["binary_feats"])
    in_maps = [
        {"local": local[c], "binary": binary[c], **shared} for c in range(NCORES)
    ]
    res = run_bass_kernel_spmd(
        nc, in_maps, core_ids=list(range(NCORES)), trace=trace
    )
    lp = np.stack([r["out_lp"] for r in res.results])
    gp = np.stack([r["out_gp"] for r in res.results])
    return (lp, gp), res


def kernel(**inputs):
    out, _ = _run(inputs, trace=False)
    return out



# revision 2
# speedup vs baseline: 1.2763x; 1.2763x over previous
"""Trainium2 Bass kernel for nn_Attention_54580444397738 (gnn_message_passing) v2.

Math per batch b (B=8, N=128, H=256, C=16):
  proj         = local @ W_apair                                     [N, H]
  pre[i,j,:]   = proj[i,:] + proj[j,:] + binary[i,j,:] @ W_binary
                 + b_apair + b_binary                                [N, N, H]
  score[i,j]   = sigmoid(relu(pre[i,j,:]) . W_att + b_att)           [N, N]
  glob         = score @ local                                       [N, H]
  local_pair [i,j,:] = local[i,:] + local[j,:]                       (output 1)
  global_pair[i,j,:] = glob[i,:]  + glob[j,:]                        (output 2)

Key design points vs the previous version:
  - Outputs are written DEVICE-SIDE as bf16 and upcast to f32 on the host
    (gate is rel<2e-2; bf16 rounding is ~2e-3 through a sum of two rounded
    terms).  HBM write traffic halves: 2 x 8.4 MB per core, ~47 us floor.
  - Output staging uses partition=i, free=(j-block of 16, h) tiles so each
    stage DMA has a contiguous 8 KB descriptor per partition (vs 1 KB
    strided in the old layout).
  - |W_att| is folded into W_apair/W_binary/biases HOST-side and the k
    columns are permuted so sign(W_att) is [+]*P ++ [-]*(256-P).  The
    attention dot then becomes two segmented DVE tensor_reduce ops per
    8-i chunk (out[128,8] from [128,8,P] view) instead of 256 per-i
    affine_mul_reduce calls: ~39 us of DVE instead of ~108 us.
  - Output tiles out[i,(jj,h)] = x[i,h](free-bcast) + x[jj,h](partition-
    bcast): variant 'G' = gpsimd partition_broadcast + one wide bf16 DVE
    add (2x mode); variant 'E' = all-PE compose in PSUM (identB @ x-bcast
    + ones-row @ flat-rows) + ACT cast-copy, balancing engines.
  - No SWDGE anywhere (casts via compute-engine copies; bf16 flat rows
    bounce through DRAM with plain HWDGE loads).
"""

import numpy as np

B, N, H, BIN = 8, 128, 256, 16
NCORES = 8
CPAD = 32        # c dim padded 16 -> 32 so transposed blocks land 32-aligned
IG = 4           # i's per binary-transpose group (4 * 32 = 128)
JS = 16          # j's per output stage tile
NSTAGE = N // JS

LP_PAT = "GGEGGEGG"   # output-stage variants, local_pair phase
GP_PAT = "EEGEEGEE"   # global_pair phase (PE/ACT are free of attention here)
SKIP_ATTN = False     # probe knob: drop attention/score work (wrong gp)
SKIP_OUT = False      # probe knob: drop output stages (no lp/gp writes)

_cache = {}


def _body(tc, io, P, reps=1):
    import concourse.bass as bass
    import concourse.mybir as mybir
    from concourse.masks import make_identity
    from contextlib import ExitStack, nullcontext

    nc = tc.nc
    ts = bass.ts
    f32 = mybir.dt.float32
    f32r = mybir.dt.float32r
    bf16 = mybir.dt.bfloat16
    Relu = mybir.ActivationFunctionType.Relu
    Sigmoid = mybir.ActivationFunctionType.Sigmoid
    AX = mybir.AxisListType.X
    ADD = mybir.AluOpType.add

    local_d, binary_d, wap_d, wbias_d, batt_d, lp_d, gp_d = io

    ctx = ExitStack()
    with ctx:
        persist = ctx.enter_context(tc.tile_pool(name="persist", bufs=1))
        binTp = ctx.enter_context(tc.tile_pool(name="binTp", bufs=4))
        a2p = ctx.enter_context(tc.tile_pool(name="a2p", bufs=3))
        stagep = ctx.enter_context(tc.tile_pool(name="stagep", bufs=3))
        bcastp = ctx.enter_context(tc.tile_pool(name="bcastp", bufs=2))
        flatp = ctx.enter_context(tc.tile_pool(name="flatp", bufs=1))
        prep = ctx.enter_context(tc.tile_pool(name="prep", bufs=3, space="PSUM"))
        pep = ctx.enter_context(tc.tile_pool(name="pep", bufs=2, space="PSUM"))
        tpp = ctx.enter_context(tc.tile_pool(name="tpp", bufs=2, space="PSUM"))

        # timing builds wrap the whole body in a device-side loop
        loop = tc.For_i(0, reps, 1) if reps > 1 else nullcontext()
        ctx.enter_context(loop)

        # ---------------- persistent setup ----------------
        # local + bf16 copy + flat rows first: the lp output pipeline
        # (bcast/add/DMA) only depends on these and starts immediately
        localSb = persist.tile([N, H], f32, tag="localSb")
        nc.sync.dma_start(out=localSb, in_=local_d)
        xbL = persist.tile([N, H], bf16, tag="xbL")
        nc.vector.tensor_copy(out=xbL, in_=localSb)

        flL = flatp.tile([1, N * H], bf16, tag="flat")
        nc.sync.dma_start(out=flL, in_=xbL)

        # Attention runs j-major: pre2_j[i,k] = proj[j,k] + proj[i,k]
        # + binary[i,j,:] @ W_binary[:,k].  binary loads CONTIGUOUSLY as
        # [i, (j,c)] (8KB descriptors vs 64B for the transposed gather),
        # then a strided DVE copy pads c 16->32 so the PE transposes of
        # 128-col groups give [(j%4, c32), i] weight tiles, 32-aligned.
        binRaw = persist.tile([128, N * BIN], f32, tag="binRaw")
        nc.scalar.dma_start(out=binRaw, in_=binary_d.rearrange("i j c -> i (j c)"))
        binp = persist.tile([128, N * CPAD], f32, tag="binp")
        binp3 = binp.rearrange("p (j c) -> p j c", c=CPAD)
        nc.vector.memset(binp3[:, :, 16:17], 1.0)
        nc.scalar.copy(
            out=binp3[:, :, 0:BIN],
            in_=binRaw.rearrange("p (j c) -> p j c", c=BIN))

        identity = persist.tile([128, 128], f32, tag="identity")
        make_identity(nc, identity)
        identB = persist.tile([128, 128], bf16, tag="identB")
        nc.scalar.copy(out=identB, in_=identity)
        onesT = persist.tile([128, 128], bf16, tag="onesT")
        nc.vector.memset(onesT, 1.0)
        onesB = persist.tile([1, 128], bf16, tag="onesB")
        nc.vector.memset(onesB, 1.0)

        # f32 loads, converted to f32r by compute-engine copies (no SWDGE)
        wapF = persist.tile([128, 2 * H], f32, tag="wapF")
        nc.scalar.dma_start(out=wapF[:, 0:H], in_=wap_d[0:128])
        nc.scalar.dma_start(out=wapF[:, H : 2 * H], in_=wap_d[128:256])
        wapR = persist.tile([128, 2 * H], f32r, tag="wapR")
        nc.scalar.copy(out=wapR, in_=wapF)

        wbiasF = persist.tile([17, H], f32, tag="wbiasF")
        nc.scalar.dma_start(out=wbiasF, in_=wbias_d)
        wbiasB = persist.tile([17, H], bf16, tag="wbiasB")
        nc.scalar.copy(out=wbiasB, in_=wbiasF)
        wx4 = persist.tile([128, H], bf16, tag="wx4")
        for m in range(4):
            nc.scalar.dma_start(out=wx4[32 * m : 32 * m + 17, :], in_=wbiasB)

        battRow = persist.tile([1, 1], f32, tag="battRow")
        nc.scalar.dma_start(out=battRow, in_=batt_d.unsqueeze(0))
        battCol = persist.tile([128, 1], f32, tag="battCol")
        nc.gpsimd.partition_broadcast(battCol, battRow)

        # localT = local^T (f32r), then projW = local @ W_apair' (f32r)
        localT = persist.tile([128, H], f32r, tag="localT")
        for hb in range(2):
            tp = tpp.tile([128, 128], f32, tag="tp")
            nc.tensor.transpose(tp, localSb[:, ts(hb, 128)], identity)
            nc.scalar.copy(out=localT[:, ts(hb, 128)], in_=tp)
        pp = prep.tile([128, 2 * H], f32, tag="pre")
        nc.tensor.matmul(pp[:, 0:H], lhsT=localT[:, 0:128], rhs=wapR[:, 0:H],
                         start=True, stop=False)
        nc.tensor.matmul(pp[:, 0:H], lhsT=localT[:, 128:256], rhs=wapR[:, H : 2 * H],
                         start=False, stop=True)
        projWb = persist.tile([128, H], bf16, tag="projWb")
        nc.scalar.copy(out=projWb, in_=pp[:, 0:H])

        # proj rows (bf16) flattened to partitions {0,32,64,96} for the
        # row-matmuls -- one combined SBUF->SBUF gather
        projFlat = persist.tile([128, 32 * H], bf16, tag="projFlat")
        nc.sync.dma_start(
            out=projFlat.rearrange("(a x) f -> a x f", x=32)[:, 0, :],
            in_=projWb)




        sp = persist.tile([128, N], f32, tag="sp")
        sm = persist.tile([128, N], f32, tag="sm")
        logitsT = persist.tile([128, N], f32, tag="logitsT")
        binT = {}

        # ---------------- helpers ----------------
        a2tiles = {}

        def attn_chunk(c):
            """pre2/relu for j in [8c, 8c+8) (PE/ACT only; the dot-reduces
            are emitted separately, one stage later, so the DVE FIFO never
            stalls the output-stage adds on fresh a2 production).

            pre2_j[i, (m,k)] accumulates in PSUM: proj[j+m] row-bcast +
            identR @ projW broadcast (adds proj[i,k] to both halves) +
            binT-group MMs (binary term + bias via the ones lane)."""
            a2 = a2p.tile([128, 8 * H], f32, tag="a2")
            a2tiles[c] = a2
            for jj in range(0, 8, 2):
                j = 8 * c + jj
                g, jl = divmod(j, IG)
                if jl == 0:  # transpose group g: [i,(j4,c32)] -> [(j4,c32),i]
                    tp = tpp.tile([128, 128], f32, tag="tp")
                    nc.tensor.transpose(tp, binp[:, ts(g, 128)], identity)
                    bt = binTp.tile([128, 128], bf16, tag="binT")
                    nc.scalar.copy(out=bt, in_=tp)
                    binT[g] = bt
                pre = prep.tile([128, 2 * H], f32, tag="pre")
                q, r = divmod(j, 32)
                nc.tensor.matmul(pre, lhsT=onesT[32 * q : 32 * q + 1, :],
                                 rhs=projFlat[32 * q : 32 * q + 1, r * H : r * H + 2 * H],
                                 start=True, stop=False, tile_position=(32 * q, 0))
                for m in range(2):
                    gg, jjl = divmod(j + m, IG)
                    nc.tensor.matmul(pre[:, ts(m, H)], lhsT=identB, rhs=projWb,
                                     start=False, stop=False)
                    nc.tensor.matmul(
                        pre[:, ts(m, H)],
                        lhsT=binT[gg][32 * jjl : 32 * jjl + 17, :],
                        rhs=wx4[32 * jjl : 32 * jjl + 17, :],
                        start=False, stop=(m == 1), tile_position=(32 * jjl, 0),
                    )
                nc.scalar.activation(out=a2[:, jj * H : (jj + 2) * H], in_=pre,
                                     func=Relu)

        def attn_reduce(c):
            a2 = a2tiles.pop(c)
            a3 = a2.rearrange("p (g k) -> p g k", k=H)
            if P > 0:
                nc.vector.tensor_reduce(out=sp[:, ts(c, 8)], in_=a3[:, :, 0:P],
                                        axis=AX, op=ADD)
            if P < H:
                nc.vector.tensor_reduce(out=sm[:, ts(c, 8)], in_=a3[:, :, P:H],
                                        axis=AX, op=ADD)

        def load_flat_all(xb):
            # one SBUF->SBUF partition->free gather per phase onto partition
            # 0 (partition_broadcast only reads partition-0 sources); the
            # per-stage slices are free-dim offsets of this row
            fl = flatp.tile([1, N * H], bf16, tag="flat")
            nc.sync.dma_start(out=fl, in_=xb)
            return fl

        def out_stage(s, v, xb, flA, dram_out):
            fl = flA[0:1, s * JS * H : (s + 1) * JS * H]
            stage = stagep.tile([128, JS * H], bf16, tag="stage")
            st3 = stage.rearrange("p (j h) -> p j h", h=H)
            if v == "G":
                bt = bcastp.tile([128, JS * H], bf16, tag="bt")
                # bitcast bf16 pairs to f32: partition_broadcast cost scales
                # with element count, so this halves the GPSIMD time
                nc.gpsimd.partition_broadcast(bt.bitcast(f32), fl.bitcast(f32))
                nc.vector.tensor_add(
                    out=st3, in0=xb.unsqueeze(1).broadcast_to([128, JS, H]),
                    in1=bt.rearrange("p (j h) -> p j h", h=H))
            else:  # 'E': all-PE compose in PSUM + ACT cast-copy
                xb2 = xb.unsqueeze(1).broadcast_to([128, 2, H])
                for p in range(8):
                    po = pep.tile([128, 2 * H], f32, tag="pe")
                    nc.tensor.matmul(po, lhsT=identB, rhs=xb2,
                                     start=True, stop=False)
                    nc.tensor.matmul(po, lhsT=onesB,
                                     rhs=fl[0:1, p * 2 * H : (p + 1) * 2 * H],
                                     start=False, stop=True)
                    nc.scalar.copy(out=stage[:, p * 2 * H : (p + 1) * 2 * H], in_=po)
            nc.sync.dma_start(out=dram_out[:, ts(s, JS), :], in_=st3)

        # ---------------- phase 1: local_pair + attention ----------------
        for s in range(NSTAGE):
            if not SKIP_OUT:
                out_stage(s, LP_PAT[s], xbL, flL, lp_d)
            if not SKIP_ATTN:
                attn_chunk(2 * s)
                attn_chunk(2 * s + 1)
                if s > 0:
                    attn_reduce(2 * (s - 1))
                    attn_reduce(2 * (s - 1) + 1)
        if not SKIP_ATTN:
            attn_reduce(2 * (NSTAGE - 1))
            attn_reduce(2 * (NSTAGE - 1) + 1)

        # ---------------- scores -> glob ----------------
        xbG = persist.tile([N, H], bf16, tag="xbG")
        if SKIP_ATTN:
            nc.vector.tensor_copy(out=xbG, in_=localSb)
        else:
            # logits/score are [i-part, j-free]; transpose for the glob MM
            score = persist.tile([128, N], f32, tag="score")
            if P == 0:
                nc.vector.tensor_scalar_mul(out=logitsT, in0=sm, scalar1=-1.0)
            elif P == H:
                nc.vector.tensor_copy(out=logitsT, in_=sp)
            else:
                nc.vector.tensor_sub(out=logitsT, in0=sp, in1=sm)
            nc.scalar.activation(out=score, in_=logitsT, func=Sigmoid,
                                 bias=battCol)
            tsc = tpp.tile([128, 128], f32, tag="tp")
            nc.tensor.transpose(tsc, score, identity)
            scoreT = persist.tile([128, N], f32, tag="scoreT")
            nc.scalar.copy(out=scoreT, in_=tsc)
            pg = prep.tile([128, 2 * H], f32, tag="pre")
            nc.tensor.matmul(pg[:, 0:H], lhsT=scoreT, rhs=localSb,
                             start=True, stop=True)
            nc.scalar.copy(out=xbG, in_=pg[:, 0:H])

        # ---------------- phase 2: global_pair ----------------
        if not SKIP_OUT:
            flG = load_flat_all(xbG)
            for s in range(NSTAGE):
                out_stage(s, GP_PAT[s], xbG, flG, gp_d)
        else:
            nc.sync.dma_start(out=gp_d[0:1, 0:1, :], in_=xbG[0:1, :])
            nc.sync.dma_start(out=lp_d[0:1, 0:1, :], in_=xbL[0:1, :])


def _build(P, reps=1):
    import concourse.bass as bass  # noqa: F401
    from concourse import bacc
    import concourse.mybir as mybir
    import concourse.tile as tile

    f32 = mybir.dt.float32
    bf16 = mybir.dt.bfloat16
    nc = bacc.Bacc(
        "TRN2",
        target_bir_lowering=False,
        debug=False,
        enable_asserts=False,
        num_devices=NCORES,
    )
    io = (
        nc.dram_tensor("local", [N, H], f32, kind="ExternalInput").ap(),
        nc.dram_tensor("binary", [N, N, BIN], f32, kind="ExternalInput").ap(),
        nc.dram_tensor("w_apair", [H, H], f32, kind="ExternalInput").ap(),
        nc.dram_tensor("wbias", [BIN + 1, H], f32, kind="ExternalInput").ap(),
        nc.dram_tensor("b_att", [1], f32, kind="ExternalInput").ap(),
        nc.dram_tensor("out_lp", [N, N, H], bf16, kind="ExternalOutput").ap(),
        nc.dram_tensor("out_gp", [N, N, H], bf16, kind="ExternalOutput").ap(),
    )
    with tile.TileContext(nc) as tc:
        _body(tc, io, P, reps=reps)
    nc.compile()
    return nc


def _prep_inputs(inputs):
    f = lambda x: np.ascontiguousarray(np.asarray(x), dtype=np.float32)
    w_att = f(inputs["W_att"]).reshape(-1)
    perm = np.argsort((w_att <= 0).astype(np.int32), kind="stable")
    P = int((w_att > 0).sum())
    a = np.abs(w_att[perm])
    wap = f(inputs["W_apair"])[:, perm] * a[None, :]
    wbin = f(inputs["W_binary"])[:, perm] * a[None, :]
    bias = (f(inputs["b_apair"]) + f(inputs["b_binary"]))[perm] * a
    wbias = np.ascontiguousarray(
        np.concatenate([wbin, bias[None, :]], axis=0), dtype=np.float32)
    shared = {
        "w_apair": np.ascontiguousarray(wap),
        "wbias": wbias,
        "b_att": f(inputs["b_att"]),
    }
    local = f(inputs["local_feats"])
    binary = f(inputs["binary_feats"])
    in_maps = [
        {"local": local[c], "binary": binary[c], **shared} for c in range(NCORES)
    ]
    return P, in_maps


def _get_nc(P):
    if P not in _cache:
        _cache[P] = _build(P)
    return _cache[P]


def _run(inputs, trace=False):
    from concourse.bass_utils import run_bass_kernel_spmd

    P, in_maps = _prep_inputs(inputs)
    nc = _get_nc(P)
    res = run_bass_kernel_spmd(
        nc, in_maps, core_ids=list(range(NCORES)), trace=trace
    )
    lp = np.stack([np.asarray(r["out_lp"]).astype(np.float32)
                   for r in res.results])
    gp = np.stack([np.asarray(r["out_gp"]).astype(np.float32)
                   for r in res.results])
    return (lp, gp), res


def kernel(**inputs):
    out, _ = _run(inputs, trace=False)
    return out
